# revision 5
# baseline (speedup 1.0000x reference)
"""Trainium2 Bass kernel: multi-head attention (B=2, T=2048, C=2048, H=16, D=128).

Sharding: tensor-parallel over heads. 8 cores x 2 heads each.
  - W_qkv columns sliced per head-pair, W_proj rows sliced per head-pair.
  - Each core computes a partial output [B*T, C]; host sums the 8 partials
    (the standard row-parallel unshard).

Per-core dataflow (no on-device transposes anywhere):
  xT [C, B*T] (host-pre-transposed) is the shared activation input.
  1) q/k proj:  lhsT = W block [c, j]  (stationary), rhs = xT [c, r]
                -> qT/kT in [d, tokens] layout (PSUM), RoPE applied on the way
                to SBUF.
  2) v proj:    lhsT = xT block [c, r] (stationary), rhs = Wv [c, j]
                -> v in [tokens, d] layout.
  3) attention (ScalarE-bound, so everything else is spread across engines):
     - mm1 pair (h0,h1) -> one [128,1024] fp32 PSUM chunk (2 banks)
     - ONE exp per chunk on ScalarE ((1024+352)/1.2 ns, amortizes overhead)
     - softmax denominator: e-chunks 1..11 accumulated into dacc on
       DVE/GpSimd (bf16, 2x mode on DVE); chunks 12..15 folded directly
       into the PE dsum matmul chain (ones stationary); dsums col-group
       packed 4-per-bank so ONE batched DVE reciprocal serves 4 groups.
     - mm2 accumulates yT[d, qi] per head; y banks freed promptly by an
       unnormalized copy to SBUF; normalize in-place after broadcast.
  4) out proj:  lhsT = yT block [j, r] (stationary), rhs = Wp [j, o]
                -> partial out [tokens, C], copies split Scalar/DVE,
                DMA'd out per 128-row block.
"""

import math

import numpy as np

N_CORES = 8
B, T, C = 2, 2048, 2048
N_HEAD, D = 16, 128
HPC = N_HEAD // N_CORES          # heads per core
JC = HPC * D                     # per-core slice width of qkv/proj dims

RT = 512                         # q tile (moving free dim) in attention
KB = 128                         # key block (contraction tile) in attention

# how many e-chunks per qt go through the PE dsum chain instead of
# elementwise dacc accumulation (they cost PE slack, save DVE/GpSimd)
N_TAIL = 4

PHASE_MARKS = []


def _build(Bp, Tp, Cp, hpc, d):
    """Build the per-core Bass graph. All cores run the same graph on
    different weight slices."""
    PHASE_MARKS.clear()
    import concourse.bacc as bacc
    import concourse.tile as tile
    from concourse import mybir

    f32 = mybir.dt.float32
    f32r = mybir.dt.float32r
    bf16 = mybir.dt.bfloat16
    Exp = mybir.ActivationFunctionType.Exp

    jc = hpc * d
    BT = Bp * Tp
    n_ck = Cp // 128             # contraction chunks for proj
    n_kb = Tp // KB              # key blocks per batch
    n_qt = Tp // RT              # query tiles per batch
    n_rb = Tp // 128             # row blocks for out proj
    n_ot = Cp // RT              # output column tiles
    scale = 1.0 / math.sqrt(d)

    nc = bacc.Bacc("TRN2", target_bir_lowering=False, debug=False)

    xT = nc.declare_dram_parameter("xT", [Cp, BT], bf16, isOutput=False)
    wqkv = nc.declare_dram_parameter("wqkv", [Cp, 3 * jc], bf16,
                                     isOutput=False)
    wp = nc.declare_dram_parameter("wp", [jc, Cp], bf16, isOutput=False)
    ones_d = nc.declare_dram_parameter("ones", [128, 128], f32r, isOutput=False)
    cosT = nc.declare_dram_parameter("cosT", [d, Tp], bf16, isOutput=False)
    sinT = nc.declare_dram_parameter("sinT", [d, Tp], bf16, isOutput=False)
    out = nc.declare_dram_parameter("out", [BT, Cp], bf16, isOutput=True)

    with tile.TileContext(nc) as tc:
        with (
            nc.allow_low_precision(reason="bf16 accumulation paths validated "
                                   "against the fp32 reference"),
            tc.tile_pool(name="wpool", bufs=1) as wpool,
            tc.tile_pool(name="acts", bufs=1) as acts,
            tc.tile_pool(name="xpool", bufs=20) as xpool,
            tc.tile_pool(name="rope", bufs=4) as rope,
            tc.tile_pool(name="epool", bufs=12) as epool,
            tc.tile_pool(name="dpool", bufs=2) as dpool,
            tc.tile_pool(name="small", bufs=2) as small,
            tc.tile_pool(name="bcpool", bufs=4) as bcpool,
            tc.tile_pool(name="opool", bufs=12) as opool,
        ):
            # ---- resident weights / tables ----
            RP = 256
            wq_sb, wk_sb, wv_sb = [], [], []
            xt_pre = []
            for ck in range(n_ck):
                t = wpool.tile([128, 3 * jc], bf16, tag=f"w{ck}",
                               name=f"w{ck}")
                nc.sync.dma_start(t, wqkv[ck * 128:(ck + 1) * 128, :])
                wq_sb.append(t[:, 0:jc])
                wk_sb.append(t[:, jc:2 * jc])
                wv_sb.append(t[:, 2 * jc:3 * jc])
                xp = xpool.tile([128, RP], bf16, tag="xt", name=f"xtpre{ck}")
                nc.sync.dma_start(xp, xT[ck * 128:(ck + 1) * 128, 0:RP])
                xt_pre.append(xp)
            cos_sb = wpool.tile([d, Tp], bf16, tag="cos")
            sin_sb = wpool.tile([d, Tp], bf16, tag="sin")
            nc.sync.dma_start(cos_sb, cosT[:])
            nc.sync.dma_start(sin_sb, sinT[:])
            ones_sb = wpool.tile([128, 1], f32r, tag="ones")
            nc.sync.dma_start(ones_sb, ones_d[:, 0:1])
            ones_bf = wpool.tile([128, 1], bf16, tag="ones_bf")
            nc.vector.tensor_copy(out=ones_bf, in_=ones_sb)
            wp_sb = wpool.tile([128, hpc, Cp], bf16, tag="wp")

            for b in range(Bp):
                qT_sb = acts.tile([128, hpc, Tp], bf16, tag="qT")
                kT_sb = acts.tile([128, hpc, Tp], bf16, tag="kT")
                v_sb = acts.tile([128, n_kb, jc], bf16, tag="v")
                yT_sb = acts.tile([128, hpc, Tp], bf16, tag="yT")

                # ================= qkv projection =================
                PHASE_MARKS.append((f"proj{b}", nc.next_id()))
                n_sub = RP // 128
                with tc.tile_pool(name="ps_proj", bufs=2, space="PSUM") as psp:
                    for rt in range(Tp // RP):
                        rsl = slice(b * Tp + rt * RP, b * Tp + (rt + 1) * RP)
                        tsl = slice(rt * RP, (rt + 1) * RP)
                        q_ps = psp.tile([128, hpc * RP], f32, tag="qps")
                        k_ps = psp.tile([128, hpc * RP], f32, tag="kps")
                        v_ps = psp.tile([128, n_sub * jc], f32, tag="vps", bufs=1)
                        for ck in range(n_ck):
                            if b == 0 and rt == 0:
                                xt = xt_pre[ck]
                            else:
                                xt = xpool.tile([128, RP], bf16, tag="xt")
                                nc.sync.dma_start(
                                    xt, xT[ck * 128:(ck + 1) * 128, rsl])
                            first = ck == 0
                            last = ck == n_ck - 1
                            for h in range(hpc):
                                nc.tensor.matmul(
                                    q_ps[:, h * RP:(h + 1) * RP],
                                    wq_sb[ck][:, h * d:(h + 1) * d],
                                    xt, start=(first and h == 0),
                                    stop=(last and h == hpc - 1),
                                    skip_group_check=True)
                                nc.tensor.matmul(
                                    k_ps[:, h * RP:(h + 1) * RP],
                                    wk_sb[ck][:, h * d:(h + 1) * d],
                                    xt, start=(first and h == 0),
                                    stop=(last and h == hpc - 1),
                                    skip_group_check=True)
                            for s in range(n_sub):
                                nc.tensor.matmul(
                                    v_ps[:, s * jc:(s + 1) * jc],
                                    xt[:, s * 128:(s + 1) * 128],
                                    wv_sb[ck], start=(first and s == 0),
                                    stop=(last and s == n_sub - 1),
                                    skip_group_check=True)
                        # rope epilogue: dst = psum*cos + swap(psum)*sin_signed
                        hd = d // 2
                        for h in range(hpc):
                            for ps, dst in (
                                (q_ps[:, h * RP:(h + 1) * RP], qT_sb),
                                (k_ps[:, h * RP:(h + 1) * RP], kT_sb),
                            ):
                                t1 = rope.tile([d, RP], f32, tag="t1")
                                nc.vector.tensor_mul(t1, ps, cos_sb[:, tsl])
                                t2 = rope.tile([d, RP], f32, tag="t2")
                                nc.vector.tensor_mul(
                                    t2[0:hd], ps[hd:d], sin_sb[0:hd, tsl])
                                nc.vector.tensor_mul(
                                    t2[hd:d], ps[0:hd], sin_sb[hd:d, tsl])
                                nc.vector.tensor_add(dst[:, h, tsl], t1, t2)
                        for s in range(n_sub):
                            nc.any.tensor_copy(
                                out=v_sb[:, rt * n_sub + s, :],
                                in_=v_ps[:, s * jc:(s + 1) * jc])

                # ================= attention =================
                # ScalarE does one exp per [128,1024] chunk (both heads,
                # one kb block).  PSUM: s 2x2 banks, y 2 banks, dsum 1.
                PHASE_MARKS.append((f"attn{b}", nc.next_id()))
                if b == 0:
                    nc.sync.dma_start(
                        wp_sb, wp.rearrange("(h p) o -> p h o", p=128))
                with (
                    tc.tile_pool(name="ps_s", bufs=2, space="PSUM") as ps_s,
                    tc.tile_pool(name="ps_y", bufs=2, space="PSUM") as ps_y,
                    tc.tile_pool(name="ps_d", bufs=1, space="PSUM") as ps_d,
                ):
                    def mm1pair(qt, j):
                        qsl = slice(qt * RT, (qt + 1) * RT)
                        s_ps = ps_s.tile([128, hpc * RT], f32, tag="s",
                                         name=f"sps{qt}_{j}")
                        for h in range(hpc):
                            nc.tensor.matmul(
                                s_ps[:, h * RT:(h + 1) * RT],
                                kT_sb[:, h, j * KB:(j + 1) * KB],
                                qT_sb[:, h, qsl],
                                start=True, stop=True,
                                skip_group_check=True)
                        return s_ps

                    n_acc = n_kb - N_TAIL   # chunks accumulated elementwise
                    state = {}              # per live qt: (ys, dacc, tail_e)

                    def start_qt(qt):
                        ys = [ps_y.tile([d, RT], f32, tag="y",
                                        name=f"yps{qt}_{h}")
                              for h in range(hpc)]
                        return (ys, None, [])

                    def step(qt, j, s_ps):
                        """exp + dacc + mm2 for chunk (qt, j)."""
                        ys, dacc, tail_e = state[qt]
                        if j == 0:
                            e_j = dpool.tile([128, hpc * RT], bf16, tag="dacc",
                                             name=f"dacc{qt}")
                            state[qt] = (ys, e_j, tail_e)
                        else:
                            e_j = epool.tile([128, hpc * RT], bf16, tag="e",
                                             name=f"e{qt}_{j}")
                        nc.scalar.activation(e_j, s_ps, Exp, scale=scale)
                        if 0 < j < n_acc:
                            dacc = state[qt][1]
                            if j % 3 == 2:
                                nc.gpsimd.tensor_add(dacc, dacc, e_j)
                            else:
                                nc.vector.tensor_add(dacc, dacc, e_j)
                        elif j >= n_acc:
                            tail_e.append(e_j)
                        for h in range(hpc):
                            nc.tensor.matmul(
                                ys[h],
                                v_sb[:, j, h * d:(h + 1) * d],
                                e_j[:, h * RT:(h + 1) * RT],
                                start=(j == 0), stop=(j == n_kb - 1),
                                skip_group_check=True)

                    def finish_qt(qt, dsum_ps):
                        """dsum chains + unnormalized y copy for qt."""
                        ys, dacc, tail_e = state.pop(qt)
                        for h in range(hpc):
                            g = 32 * ((qt % 2) * hpc + h)
                            hs = slice(h * RT, (h + 1) * RT)
                            terms = [dacc] + tail_e
                            for i, tm in enumerate(terms):
                                nc.tensor.matmul(
                                    dsum_ps[g:g + 1, :],
                                    ones_bf, tm[:, hs],
                                    start=(i == 0),
                                    stop=(i == len(terms) - 1),
                                    skip_group_check=True,
                                    tile_position=(0, g))
                        qsl = slice(qt * RT, (qt + 1) * RT)
                        for h in range(hpc):
                            nc.vector.tensor_copy(out=yT_sb[:, h, qsl],
                                                  in_=ys[h])

                    def finalize_pair(qt_pair, dsum_ps):
                        """recip + broadcast + in-place normalize for the two
                        qt's packed into dsum_ps (rows 0/32 qt even, 64/96
                        qt odd).  partition_broadcast only honors partition 0,
                        so each row is staged there with a tiny SBUF DMA."""
                        r_sb = small.tile([128, RT], f32, tag="recip",
                                          name=f"recip{qt_pair}")
                        nc.vector.reciprocal(r_sb[0:97, :], dsum_ps[0:97, :])
                        for qt in (2 * qt_pair, 2 * qt_pair + 1):
                            qsl = slice(qt * RT, (qt + 1) * RT)
                            for h in range(hpc):
                                g = 32 * ((qt % 2) * hpc + h)
                                st = bcpool.tile([1, RT], f32, tag="stage",
                                                 name=f"st{qt}_{h}")
                                nc.sync.dma_start(st[0:1, :],
                                                  r_sb[g:g + 1, :])
                                bc = bcpool.tile([128, RT], f32, tag="bc",
                                                 name=f"bc{qt}_{h}")
                                nc.gpsimd.partition_broadcast(
                                    out_ap=bc, in_ap=st[0:1, :])
                                nc.vector.tensor_mul(
                                    yT_sb[:, h, qsl], yT_sb[:, h, qsl], bc)

                    # steady-state loop with 2-chunk mm1 lookahead.  The
                    # recip/broadcast/normalize group of each qt-pair is
                    # deferred into the middle of the NEXT qt's stream so it
                    # never delays the dacc adds that recycle e-pool buffers.
                    state[0] = start_qt(0)
                    pend = [(0, 0, mm1pair(0, 0)), (0, 1, mm1pair(0, 1))]
                    dsum_ps = None
                    deferred = None
                    for qt in range(n_qt):
                        if qt % 2 == 0:
                            dsum_ps = ps_d.tile([128, RT], f32, tag="dsum",
                                                name=f"dsum{qt // 2}")
                        for j in range(n_kb):
                            cqt, cj, s_ps = pend.pop(0)
                            assert (cqt, cj) == (qt, j)
                            step(qt, j, s_ps)
                            if j == 6 and deferred is not None:
                                finalize_pair(*deferred)
                                deferred = None
                            nj = j + 2
                            if nj < n_kb:
                                pend.append((qt, nj, mm1pair(qt, nj)))
                            elif qt + 1 < n_qt:
                                if nj - n_kb == 0:
                                    state[qt + 1] = start_qt(qt + 1)
                                pend.append((qt + 1, nj - n_kb,
                                             mm1pair(qt + 1, nj - n_kb)))
                        finish_qt(qt, dsum_ps)
                        if qt % 2 == 1:
                            deferred = (qt // 2, dsum_ps)
                    finalize_pair(*deferred)

                # ============== output projection ==============
                PHASE_MARKS.append((f"outproj{b}", nc.next_id()))
                with tc.tile_pool(name="ps_o", bufs=2, space="PSUM") as ps_o:
                    for rb in range(n_rb):
                        for ot in range(n_ot):
                            o_ps = ps_o.tile([128, RT], f32, tag="ops")
                            for h in range(hpc):
                                nc.tensor.matmul(
                                    o_ps,
                                    yT_sb[:, h, rb * 128:(rb + 1) * 128],
                                    wp_sb[:, h, ot * RT:(ot + 1) * RT],
                                    start=(h == 0), stop=(h == hpc - 1))
                            o_sb = opool.tile([128, RT], bf16, tag="o")
                            if ot % 2 == 0:
                                nc.vector.tensor_copy(out=o_sb, in_=o_ps)
                            else:
                                nc.scalar.activation(
                                    o_sb, o_ps,
                                    mybir.ActivationFunctionType.Copy)
                            nc.sync.dma_start(
                                out[b * Tp + rb * 128:b * Tp + (rb + 1) * 128,
                                    ot * RT:(ot + 1) * RT],
                                o_sb)

    PHASE_MARKS.append(("tail", nc.next_id()))
    nc.compile()
    return nc


def _prep_in_maps(x, cos, sin, W_qkv, W_proj, n_cores, hpc, d):
    """Host-side shard prep: pure layout work (transpose / slice / sign fold)."""
    Bp, Tp, Cp = x.shape
    jc = hpc * d
    import ml_dtypes
    xTa = np.ascontiguousarray(x.reshape(Bp * Tp, Cp).T).astype(ml_dtypes.bfloat16)
    cosT = np.ascontiguousarray(cos.T).astype(ml_dtypes.bfloat16)
    sinT = np.ascontiguousarray(sin.T).copy()
    sinT[: d // 2] *= -1.0
    sinT = sinT.astype(ml_dtypes.bfloat16)
    in_maps = []
    for c in range(n_cores):
        j0, j1 = c * jc, (c + 1) * jc
        in_maps.append({
            "xT": xTa,
            "wqkv": np.ascontiguousarray(np.concatenate(
                [W_qkv[:, j0:j1], W_qkv[:, Cp + j0:Cp + j1],
                 W_qkv[:, 2 * Cp + j0:2 * Cp + j1]], axis=1,
            )).astype(ml_dtypes.bfloat16),
            "wp": np.ascontiguousarray(W_proj[j0:j1, :]).astype(ml_dtypes.bfloat16),
            "ones": np.ones((128, 128), dtype=np.float32),
            "cosT": cosT,
            "sinT": sinT,
        })
    return in_maps


def _install_ntff_hook():
    """Enable NTFF profiling under axon when the boot image lacks the
    antenv.axon_hooks shim. Harmless if anything is missing."""
    import sys
    import types
    try:
        from antenv.axon_hooks import get_axon_ntff_profile_hook
        if get_axon_ntff_profile_hook() is not None:
            return
    except ImportError:
        pass
    try:
        sys.path.insert(0, "/root/.axon_site")
        from trn_agent_boot.trn_boot import _ntff_profile_via_ctypes

        hook = _ntff_profile_via_ctypes("/opt/axon/libaxon_pjrt.so")
        if hook is None:
            return
        mod = types.ModuleType("antenv.axon_hooks")
        mod.get_axon_ntff_profile_hook = lambda: hook
        mod.set_axon_ntff_profile_hook = lambda h: None
        import antenv
        antenv.axon_hooks = mod
        sys.modules["antenv.axon_hooks"] = mod
    except Exception:
        pass


def _run(x, cos, sin, W_qkv, W_proj, trace=False):
    from concourse.bass_utils import run_bass_kernel_spmd

    if trace:
        _install_ntff_hook()

    x = np.ascontiguousarray(x, dtype=np.float32)
    cos = np.ascontiguousarray(cos, dtype=np.float32)
    sin = np.ascontiguousarray(sin, dtype=np.float32)
    W_qkv = np.ascontiguousarray(W_qkv, dtype=np.float32)
    W_proj = np.ascontiguousarray(W_proj, dtype=np.float32)

    Bp, Tp, Cp = x.shape
    nc = _build(Bp, Tp, Cp, HPC, D)
    in_maps = _prep_in_maps(x, cos, sin, W_qkv, W_proj, N_CORES, HPC, D)
    res = run_bass_kernel_spmd(nc, in_maps, core_ids=list(range(N_CORES)),
                               trace=trace)
    acc = np.zeros((Bp * Tp, Cp), dtype=np.float32)
    for i in range(N_CORES):
        acc += np.asarray(res.results[i]["out"], dtype=np.float32)
    return acc.reshape(Bp, Tp, Cp), res


def kernel(x, cos, sin, W_qkv, W_proj):
    out, _ = _run(x, cos, sin, W_qkv, W_proj, trace=False)
    return out
